# revision 17
# baseline (speedup 1.0000x reference)
"""Multi-head attention TRN2 Bass kernel for nn_MultiHeadAttention_77610059039245.

Problem: B=4, S=2048, E=1024, H=16 heads, d_head=64, causal mask,
scale = 1/sqrt(1024). f32 inputs/outputs; internal compute mostly bf16
(f32 PSUM accumulation), with fp8 (e4m3) used where precision allows.

Sharding (8 cores): core c = (b, g): batch b = c//2, head-group g = c%2.
Each core computes heads 8g..8g+7 of batch b; host sums the two partial
output projections per batch (the Wo row-split all-reduce).

v3 changes over the 320us v2 baseline:
  - Scores matmuls in fp8 DoubleRow (0.5 PE cycles/row instead of 1.0):
    q and k are copied from the projection PSUM into [64, 2, S] fp8
    slot-pair tiles; per head one DR matmul contracts (d=64) x (2 slots)
    with the dead q slot zeroed.
  - Region-phased schedule: chunk projections for region r+1 and the
    output projection for region r-1 are interleaved BETWEEN the strips
    of region r, so the in-order PE queue has slack work while the
    Activation engine (exp is ~145us/rep, the co-bottleneck) catches up.
  - All DMA moved to the two HWDGE queues (sync=slabs, scalar=weights
    and out stores); the gpsimd SWDGE path (which burned ~39us of Pool
    engine time in descriptor generation) is no longer used for bulk
    traffic.
  - PSUM->SBUF copies split between DVE and Pool to keep DVE off the
    critical path.
"""
from collections import deque

import numpy as np

import concourse.bass as bass
import concourse.mybir as mybir
import concourse.tile as tile
from concourse import bacc
from concourse.bass_utils import run_bass_kernel_spmd

F32 = mybir.dt.float32
BF16 = mybir.dt.bfloat16
F8 = mybir.dt.float8e4
DR = mybir.MatmulPerfMode.DoubleRow
EXP = mybir.ActivationFunctionType.Exp

B, S, E, H = 4, 2048, 1024, 16
D = 64                    # head dim
HC = 8                    # heads per core
HP = HC // 2              # head pairs per core
GD = HC * D               # per-core projected width (512)
SCALE = 1.0 / 32.0 / 256.0   # 1/sqrt(QK=1024), /256 for the 16x
                             # host-prescale of Wq and Wk (fp8 range)
N_CORES = 8
CH = 512                  # projection s-chunk
NCH = S // CH             # 4
ST = S // 128             # 16 j-tiles
IT = S // 512             # 4 i-strips / regions


def build_core_kernel(reps=1):
    nc = bacc.Bacc("TRN2", target_bir_lowering=False)

    xq = nc.dram_tensor("xqT", [E, S], F8, kind="ExternalInput")
    xk = nc.dram_tensor("xkT", [E, S], F8, kind="ExternalInput")
    xv = nc.dram_tensor("xvT", [E, S], BF16, kind="ExternalInput")
    wq = nc.dram_tensor("wq", [E, GD], F8, kind="ExternalInput")
    wk = nc.dram_tensor("wk", [E, GD], F8, kind="ExternalInput")
    wv = nc.dram_tensor("wv", [E, GD], BF16, kind="ExternalInput")
    wo = nc.dram_tensor("wo", [GD, E], BF16, kind="ExternalInput")
    tril = nc.dram_tensor("tril", [128, 2, 128], BF16, kind="ExternalInput")
    out = nc.dram_tensor("out", [S, E], BF16, kind="ExternalOutput")

    with tile.TileContext(nc) as tc:
        with (
            tc.tile_pool(name="consts", bufs=1) as consts,
            tc.tile_pool(name="wpool", bufs=1) as wpool,
            tc.tile_pool(name="slab", bufs=6) as slabp,
            tc.tile_pool(name="qkv", bufs=1) as qkv,
            tc.tile_pool(name="pt", bufs=4) as ptp,
            tc.tile_pool(name="small", bufs=3) as small,
            tc.tile_pool(name="xto", bufs=1) as xtop,
            tc.tile_pool(name="ostage", bufs=4) as ostage,
            tc.tile_pool(name="ps", bufs=2, space="PSUM") as ps,
            tc.tile_pool(name="psx", bufs=2, space="PSUM") as psx,
        ):
            tril_t = consts.tile([128, 2, 128], BF16, name="tril_t")
            nc.scalar.dma_start(out=tril_t, in_=tril[:, :, :])

            # ---- persistent tensors (allocated once; memory stable) ----
            # Scores are plain bf16 K=128 matmuls (HW microbench: bf16 at
            # 223ns/matmul beats fp8-DR at 259ns for free=512; DR only pays
            # when it halves instruction count, i.e. in the projections).
            # qz[h]: [128, S] bf16, rows (h%2)*64..+64 hold q_h^T, other
            # rows ZERO (so the K=128 contraction vs the k-pair tile yields
            # exactly head h's scores).
            qz = [qkv.tile([128, S], BF16, tag=f"qz{h}", name=f"qz{h}")
                  for h in range(HC)]
            kT = [qkv.tile([128, S], BF16, tag=f"kT{p}", name=f"kT{p}")
                  for p in range(HP)]
            # v augmented with a ones column (softmax denominator from PV)
            v_aug = qkv.tile([128, HC, ST, D + 1], BF16, tag="v_aug",
                             name="v_aug")
            xT_out = [xtop.tile([128, S], BF16, tag=f"xto{p}", name=f"xto{p}")
                      for p in range(HP)]

            # one-time zero fill of the dead q halves and the ones col
            for h in range(HC):
                dead = slice(64, 128) if h % 2 == 0 else slice(0, 64)
                nc.vector.memset(qz[h][dead, :], 0.0)
            nc.gpsimd.memset(v_aug[:, :, :, D:D + 1], 1.0)

            # ---- weights: load once into SBUF (scalar HWDGE queue) ----
            def load_w(wdram, nm, dt):
                tiles = []
                for et in range(8):
                    t = wpool.tile([128, GD], dt, tag=f"w{nm}{et}",
                                   name=f"w{nm}{et}")
                    nc.scalar.dma_start(
                        out=t, in_=wdram[et * 128:(et + 1) * 128, :])
                    tiles.append(t)
                return tiles

            def load_w_paired(wdram, nm):
                # fp8 DoubleRow lhsT: [128, 2, GD], slot i = E-rows 128i
                tiles = []
                src8 = wdram.rearrange("(a p) m -> p a m", p=128)
                for t4 in range(4):
                    t = wpool.tile([128, 2, GD], F8, tag=f"w{nm}{t4}",
                                   name=f"w{nm}{t4}")
                    nc.scalar.dma_start(out=t, in_=src8[:, 2 * t4:2 * t4 + 2, :])
                    tiles.append(t)
                return tiles

            wts = {"q": load_w_paired(wq, "q"), "k": load_w_paired(wk, "k"),
                   "v": load_w(wv, "v", BF16)}
            wot = []
            for kt in range(4):
                for eh in range(2):
                    t = wpool.tile([128, GD], BF16, tag=f"wo{kt}{eh}",
                                   name=f"wo{kt}{eh}")
                    nc.scalar.dma_start(
                        out=t, in_=wo[kt * 128:(kt + 1) * 128,
                                      eh * 512:(eh + 1) * 512])
                    wot.append(t)

            # ---------------- shared emission machinery ----------------
            # (defined once; closures are rep-agnostic except via arguments)
            def transpose_chunk(xdram, sc, dt):
                slab = slabp.tile([128, 8, CH], dt, tag="slab", name="slab")
                src = xdram.rearrange("(a p) s -> p a s", p=128)
                nc.sync.dma_start(
                    out=slab[:, 0:4], in_=src[:, 0:4, sc * CH:(sc + 1) * CH])
                nc.sync.dma_start(
                    out=slab[:, 4:8], in_=src[:, 4:8, sc * CH:(sc + 1) * CH])
                return slab

            def _proj_qk_dr(wt, slab, pb):
                # DoubleRow: lhsT [128,2,128], rhs [128,2,512]; the two
                # slots are consecutive 128-row blocks of E (K=256/step)
                pj = ps.tile([128, CH], F32, tag="ps", name="pj")
                for t in range(4):
                    nc.tensor.matmul(
                        pj, wt[t][:, :, pb * 128:(pb + 1) * 128],
                        slab[:, 2 * t:2 * t + 2, :],
                        start=(t == 0), stop=(t == 3), perf_mode=DR)
                return pj

            def make_proj_units(nm, sc):
                """Create filler units for chunk (nm, sc); issues the slab
                DMA immediately (prefetch), returns per-block closures."""
                xd, dt = {"q": (xq, F8), "k": (xk, F8), "v": (xv, BF16)}[nm]
                slab = transpose_chunk(xd, sc, dt)
                units = []
                if nm in ("q", "k"):
                    wt = wts[nm]
                    for pb in range(HP):
                        def u(pb=pb, slab=slab):
                            pj = _proj_qk_dr(wt, slab, pb)
                            cs = slice(sc * CH, (sc + 1) * CH)
                            if nm == "q":
                                nc.vector.tensor_copy(
                                    qz[2 * pb][0:64, cs], pj[0:64, :])
                                nc.vector.tensor_copy(
                                    qz[2 * pb + 1][64:128, cs],
                                    pj[64:128, :])
                            else:
                                nc.vector.tensor_copy(kT[pb][:, cs], pj)
                        units.append(u)
                else:
                    wt = wts["v"]
                    for st4 in range(CH // 128):
                        def u(st4=st4, slab=slab):
                            jt = sc * (CH // 128) + st4
                            pj = ps.tile([128, GD], F32, tag="ps", name="pj")
                            for et in range(8):
                                nc.tensor.matmul(
                                    pj, slab[:, et, st4 * 128:(st4 + 1) * 128],
                                    wt[et],
                                    start=(et == 0), stop=(et == 7))
                            nc.vector.tensor_copy(
                                v_aug[:, :, jt, 0:D],
                                pj.rearrange("p (h d) -> p h d", h=HC))
                        units.append(u)
                return units

            def emit_outproj_tile(st):
                po = ps.tile([128, 2, 512], F32, tag="ps", name="po")
                for eh in range(2):
                    for kt in range(4):
                        nc.tensor.matmul(
                            po[:, eh, :],
                            xT_out[kt][:, st * 128:(st + 1) * 128],
                            wot[kt * 2 + eh],
                            start=(kt == 0), stop=(kt == 3))
                ot = ostage.tile([128, 1024], BF16, tag="ostage",
                                 name="ot")
                nc.vector.tensor_copy(ot, po.rearrange("p a b -> p (a b)"))
                nc.scalar.dma_start(
                    out=out[st * 128:(st + 1) * 128, :], in_=ot)

            def emit_scores(p, it, jt):
                kdiag = jt - 4 * it
                c0 = 128 * kdiag if kdiag > 0 else 0
                i0 = it * 512 + c0
                i1 = (it + 1) * 512
                sw = ps.tile([128, 2, 512], F32, tag="ps", name="sw")
                kt_tile = kT[p][:, jt * 128:(jt + 1) * 128]
                nc.tensor.matmul(sw[:, 0, c0:], kt_tile, qz[2 * p][:, i0:i1],
                                 start=True, stop=True)
                nc.tensor.matmul(sw[:, 1, c0:], kt_tile,
                                 qz[2 * p + 1][:, i0:i1],
                                 start=True, stop=True)
                return sw, c0

            def emit_normalize(p, it, pxp):
                rrow = small.tile([1, 2, 512], F32, tag="rrow", name="rrow")
                nc.vector.reciprocal(rrow, pxp[64:65, :, :])
                bc = small.tile([64, 2, 512], F32, tag="bc", name="bc")
                nc.gpsimd.partition_broadcast(bc, rrow)
                for hh in range(2):
                    nc.vector.tensor_mul(
                        xT_out[p][hh * 64:(hh + 1) * 64,
                                  it * 512:(it + 1) * 512],
                        pxp[0:64, hh, :], bc[:, hh, :])

            state = {"carry": None, "pending": None}
            fillerq = deque()

            def drain(n):
                while n > 0 and fillerq:
                    fillerq.popleft()()
                    n -= 1

            def drain_all():
                drain(len(fillerq))

            def emit_strip(p, it, nxt=None, seg_hook=None):
                h1, h2 = 2 * p, 2 * p + 1
                jmax = 4 * it + 3
                pxp = psx.tile([128, 2, 512], F32, tag="psx", name="pxp")
                if state["carry"] is not None:
                    sw_cur, c0_cur = state["carry"]
                    state["carry"] = None
                else:
                    sw_cur, c0_cur = emit_scores(p, it, 0)
                if state["pending"] is not None:
                    emit_normalize(*state["pending"])
                    state["pending"] = None
                for jt in range(jmax + 1):
                    if jt > 0 and jt % 2 == 0:
                        # segment boundary: inject filler PE work so the
                        # in-order PE queue has slack while ACT drains exp
                        if seg_hook is not None and jt == 4:
                            seg_hook()
                        drain(1)
                    if jt < jmax:
                        sw_next, c0_next = emit_scores(p, it, jt + 1)
                    elif nxt is not None:
                        # chain: next strip's jt=0 scores (k chunk 0 is
                        # always resident) so PE has no refill bubble
                        state["carry"] = emit_scores(nxt[0], nxt[1], 0)
                    pt = ptp.tile([128, 2, 512], BF16, tag="pt", name="pt")
                    c0 = c0_cur
                    nc.scalar.activation(pt[:, :, c0:], sw_cur[:, :, c0:],
                                         EXP, scale=SCALE)
                    kdiag = jt - 4 * it
                    if kdiag >= 0:
                        cs = slice(c0, c0 + 128)
                        nc.gpsimd.tensor_mul(pt[:, :, cs], pt[:, :, cs],
                                             tril_t)
                    nc.tensor.matmul(
                        pxp[0:65, 0, c0:], v_aug[:, h1, jt, :], pt[:, 0, c0:],
                        start=(jt == 0), stop=(jt == jmax))
                    nc.tensor.matmul(
                        pxp[0:65, 1, c0:], v_aug[:, h2, jt, :], pt[:, 1, c0:],
                        start=(jt == 0), stop=(jt == jmax))
                    if jt < jmax:
                        sw_cur, c0_cur = sw_next, c0_next
                state["pending"] = (p, it, pxp)

            # ---------------- region-phased, cross-rep-pipelined stream ----
            with nc.named_scope("proj_attn"):
                for rep in range(reps):
                    if rep == 0:
                        # cold start: q0, k0, v0 directly; q1..q3 queued
                        for u in make_proj_units("q", 0):
                            u()
                        for u in make_proj_units("k", 0):
                            u()
                        for u in make_proj_units("v", 0):
                            u()
                        for c in (1, 2):
                            fillerq.extend(make_proj_units("q", c))
                        fillerq.extend(make_proj_units("q", 3))
                    for r in range(IT):
                        # prerequisites for region r's strips were queued a
                        # region ago; force any leftovers out now
                        drain_all()
                        if r + 1 < IT:
                            fillerq.extend(make_proj_units("k", r + 1))
                            fillerq.extend(make_proj_units("v", r + 1))
                        if r >= 1:
                            for st in range(4 * (r - 1), 4 * r):
                                fillerq.append(
                                    lambda st=st: emit_outproj_tile(st))
                        if r == 0 and rep > 0:
                            # out-projection of the previous rep's region 3
                            for st in range(4 * (IT - 1), ST):
                                fillerq.append(
                                    lambda st=st: emit_outproj_tile(st))
                            # this rep's q3 (readers: last rep's strips, done)
                            fillerq.extend(make_proj_units("q", 3))
                        if r == IT - 1 and rep + 1 < reps:
                            # pull next rep's q projections into this rep's
                            # ACT-bound tail (their readers are done)
                            for c in (0, 1, 2):
                                fillerq.extend(make_proj_units("q", c))
                        for p in range(HP):
                            hook = None
                            if (r == IT - 1 and p == HP - 1
                                    and rep + 1 < reps):
                                # k0/v0 of the next rep become writable once
                                # this strip's jt 0..3 have consumed them
                                def hook():
                                    fillerq.extend(make_proj_units("k", 0))
                                    fillerq.extend(make_proj_units("v", 0))
                            nxt = None
                            if not (r == IT - 1 and p == HP - 1):
                                nxt = (p + 1, r) if p + 1 < HP else (0, r + 1)
                            emit_strip(p, r, nxt, seg_hook=hook)
                            drain(1)
                    drain_all()
                    if rep + 1 < reps:
                        pass
                    else:
                        if state["pending"] is not None:
                            emit_normalize(*state["pending"])
                            state["pending"] = None
                        for st in range(4 * (IT - 1), ST):
                            emit_outproj_tile(st)

    nc.finalize()
    return nc


_NC = None


def _get_nc():
    global _NC
    if _NC is None:
        _NC = build_core_kernel()
    return _NC


def _tril_mask_bf16():
    import ml_dtypes
    r = np.arange(128)
    m = np.where(r[:, None] <= r[None, :], 1.0, 0.0).astype(ml_dtypes.bfloat16)
    return np.ascontiguousarray(np.repeat(m[:, None, :], 2, axis=1))


def make_in_maps(query, key, value, Wq, Wk, Wv, Wo):
    import ml_dtypes
    bf = ml_dtypes.bfloat16
    f8 = ml_dtypes.float8_e4m3
    query = np.asarray(query, np.float32)
    key = np.asarray(key, np.float32)
    value = np.asarray(value, np.float32)
    # Wq/Wk prescaled x16 so fp8 e4m3 sees sigma~0.5; exp scale folds 1/256
    Wq = (np.ascontiguousarray(np.asarray(Wq, np.float32)) * 16.0).astype(f8)
    Wk = (np.ascontiguousarray(np.asarray(Wk, np.float32)) * 16.0).astype(f8)
    Wv = np.ascontiguousarray(np.asarray(Wv, np.float32)).astype(bf)
    Wo = np.ascontiguousarray(np.asarray(Wo, np.float32)).astype(bf)
    tril_m = _tril_mask_bf16()
    xTq = np.ascontiguousarray(query.transpose(0, 2, 1)).astype(f8)
    xTk = np.ascontiguousarray(key.transpose(0, 2, 1)).astype(f8)
    xTv = np.ascontiguousarray(value.transpose(0, 2, 1)).astype(bf)
    xT = [xTq, xTk, xTv]
    in_maps = []
    for c in range(N_CORES):
        b, g = c // 2, c % 2
        cols = slice(g * GD, (g + 1) * GD)
        in_maps.append({
            "xqT": xT[0][b],
            "xkT": xT[1][b],
            "xvT": xT[2][b],
            "wq": np.ascontiguousarray(Wq[:, cols]),
            "wk": np.ascontiguousarray(Wk[:, cols]),
            "wv": np.ascontiguousarray(Wv[:, cols]),
            "wo": np.ascontiguousarray(Wo[g * GD:(g + 1) * GD, :]),
            "tril": tril_m,
        })
    return in_maps


def kernel(query, key, value, mask, Wq, Wk, Wv, Wo, **run_kwargs):
    nc = _get_nc()
    in_maps = make_in_maps(query, key, value, Wq, Wk, Wv, Wo)
    res = run_bass_kernel_spmd(nc, in_maps, core_ids=list(range(N_CORES)),
                               **run_kwargs)
    out = np.empty((B, S, E), np.float32)
    for b in range(B):
        out[b] = (res.results[2 * b]["out"].astype(np.float32)
                  + res.results[2 * b + 1]["out"].astype(np.float32))
    if run_kwargs:
        kernel.last_result = res
    return out


if __name__ == "__main__":
    rng = np.random.default_rng(0)
    q = rng.standard_normal((B, S, E), dtype=np.float32)
    k = rng.standard_normal((B, S, E), dtype=np.float32)
    v = rng.standard_normal((B, S, E), dtype=np.float32)
    sc = 1.0 / np.sqrt(E)
    Wq = rng.standard_normal((E, E), dtype=np.float32) * sc
    Wk = rng.standard_normal((E, E), dtype=np.float32) * sc
    Wv = rng.standard_normal((E, E), dtype=np.float32) * sc
    Wo = rng.standard_normal((E, E), dtype=np.float32) * sc
    o = kernel(q, k, v, None, Wq, Wk, Wv, Wo)
    print("out", o.shape, o.dtype, float(np.abs(o).mean()))


# revision 18
# speedup vs baseline: 1.4339x; 1.4339x over previous
"""Multi-head attention TRN2 Bass kernel for nn_MultiHeadAttention_77610059039245.

Problem: B=4, S=2048, E=1024, H=16 heads, d_head=64, causal mask,
scale = 1/sqrt(1024). f32 inputs/outputs; internal compute in bf16
(all matmul operands), f32 PSUM accumulation.

Sharding (8 cores): core c = (b, g): batch b = c//2, head-group g = c%2.
Each core computes heads 8g..8g+7 of batch b; host sums the two partial
output projections per batch (the Wo row-split all-reduce).

Key differences from v1 (empirically motivated by HW microbenchmarks —
real matmul cost ~= 72ns + 0.25ns*N_free, with a ~3x per-row penalty for
contraction K=64):
  - All matmul operands bf16 (halves DMA; PSUM stays f32).
  - Scores use zero-padded q tiles so the contraction is K=128 (k-pair
    tile [k_h; k_h'] x [q_h; 0]) instead of two K=64 matmuls.
  - Projection chunks are 512 wide (fewer, larger matmuls).
  - Weights are loaded into SBUF once (outside the rep loop).
  - Output DMA'd directly from PSUM (no SBUF staging copy).
"""
import numpy as np

import concourse.bass as bass
import concourse.mybir as mybir
import concourse.tile as tile
from concourse import bacc
from concourse.bass_utils import run_bass_kernel_spmd

F32 = mybir.dt.float32
BF16 = mybir.dt.bfloat16
F8 = mybir.dt.float8e4
DR = mybir.MatmulPerfMode.DoubleRow
EXP = mybir.ActivationFunctionType.Exp

B, S, E, H = 4, 2048, 1024, 16
D = 64                    # head dim
HC = 8                    # heads per core
HP = HC // 2              # head pairs per core
GD = HC * D               # per-core projected width (512)
SCALE = 1.0 / 32.0 / 256.0   # 1/sqrt(QK=1024), /256 for the 16x
                             # host-prescale of Wq and Wk (fp8 range)
N_CORES = 8
CH = 512                  # projection s-chunk
NCH = S // CH             # 4
ST = S // 128             # 16 j-tiles
IT = S // 512             # 4 i-strips


def build_core_kernel(reps=1):
    nc = bacc.Bacc("TRN2", target_bir_lowering=False)

    xq = nc.dram_tensor("xqT", [E, S], F8, kind="ExternalInput")
    xk = nc.dram_tensor("xkT", [E, S], F8, kind="ExternalInput")
    xv = nc.dram_tensor("xvT", [E, S], BF16, kind="ExternalInput")
    wq = nc.dram_tensor("wq", [E, GD], F8, kind="ExternalInput")
    wk = nc.dram_tensor("wk", [E, GD], F8, kind="ExternalInput")
    wv = nc.dram_tensor("wv", [E, GD], BF16, kind="ExternalInput")
    wo = nc.dram_tensor("wo", [GD, E], BF16, kind="ExternalInput")
    tril = nc.dram_tensor("tril", [128, 2, 128], BF16, kind="ExternalInput")
    out = nc.dram_tensor("out", [S, E], BF16, kind="ExternalOutput")

    with tile.TileContext(nc) as tc:
        with (
            tc.tile_pool(name="consts", bufs=1) as consts,
            tc.tile_pool(name="wpool", bufs=1) as wpool,
            tc.tile_pool(name="slab", bufs=4) as slabp,
            tc.tile_pool(name="qkv", bufs=1) as qkv,
            tc.tile_pool(name="pt", bufs=4) as ptp,
            tc.tile_pool(name="small", bufs=3) as small,
            tc.tile_pool(name="xto", bufs=1) as xtop,
            tc.tile_pool(name="ostage", bufs=4) as ostage,
            tc.tile_pool(name="ps", bufs=2, space="PSUM") as ps,
            tc.tile_pool(name="psx", bufs=2, space="PSUM") as psx,
        ):
            tril_t = consts.tile([128, 2, 128], BF16, name="tril_t")
            nc.sync.dma_start(out=tril_t, in_=tril[:, :, :])

            # ---- persistent tensors (allocated once; memory stable) ----
            # qz[h]: [128, S] bf16, rows (h%2)*64..+64 hold q_h^T, other
            # rows ZERO (so the K=128 scores contraction vs the k-pair
            # tile yields exactly head h's scores).
            qz = [qkv.tile([128, S], BF16, tag=f"qz{h}", name=f"qz{h}")
                  for h in range(HC)]
            kT = [qkv.tile([128, S], BF16, tag=f"kT{p}", name=f"kT{p}")
                  for p in range(HP)]
            # v augmented with a ones column (softmax denominator from PV)
            v_aug = qkv.tile([128, HC, ST, D + 1], BF16, tag="v_aug",
                             name="v_aug")
            xT_out = [xtop.tile([128, S], BF16, tag=f"xto{p}", name=f"xto{p}")
                      for p in range(HP)]

            # one-time zero fill of the pad halves of qz and the ones col
            for h in range(HC):
                dead = slice(64, 128) if h % 2 == 0 else slice(0, 64)
                nc.vector.memset(qz[h][dead, :], 0.0)
            nc.gpsimd.memset(v_aug[:, :, :, D:D + 1], 1.0)

            # ---- weights: load once into SBUF ----
            def load_w(wdram, nm, dt):
                tiles = []
                for et in range(8):
                    t = wpool.tile([128, GD], dt, tag=f"w{nm}{et}",
                                   name=f"w{nm}{et}")
                    eng = nc.sync if et % 2 == 0 else nc.gpsimd
                    eng.dma_start(out=t, in_=wdram[et * 128:(et + 1) * 128, :])
                    tiles.append(t)
                return tiles

            def load_w_paired(wdram, nm):
                # fp8 DoubleRow lhsT: [128, 2, GD], slot i = E-rows 128i
                tiles = []
                src8 = wdram.rearrange("(a p) m -> p a m", p=128)
                for t4 in range(4):
                    t = wpool.tile([128, 2, GD], F8, tag=f"w{nm}{t4}",
                                   name=f"w{nm}{t4}")
                    eng = nc.sync if t4 % 2 == 0 else nc.gpsimd
                    eng.dma_start(out=t, in_=src8[:, 2 * t4:2 * t4 + 2, :])
                    tiles.append(t)
                return tiles

            wts = {"q": load_w_paired(wq, "q"), "k": load_w_paired(wk, "k"),
                   "v": load_w(wv, "v", BF16)}
            wot = []
            for kt in range(4):
                for eh in range(2):
                    t = wpool.tile([128, GD], BF16, tag=f"wo{kt}{eh}",
                                   name=f"wo{kt}{eh}")
                    nc.gpsimd.dma_start(
                        out=t, in_=wo[kt * 128:(kt + 1) * 128,
                                      eh * 512:(eh + 1) * 512])
                    wot.append(t)

            for _rep in range(reps):
                # ---------------- projections ----------------
                def transpose_chunk(xdram, sc, dt=BF16):
                    slab = slabp.tile([128, 8, CH], dt, tag="slab",
                                      name="slab")
                    src = xdram.rearrange("(a p) s -> p a s", p=128)
                    nc.gpsimd.dma_start(
                        out=slab[:, 0:4], in_=src[:, 0:4, sc * CH:(sc + 1) * CH])
                    nc.sync.dma_start(
                        out=slab[:, 4:8], in_=src[:, 4:8, sc * CH:(sc + 1) * CH])
                    return slab

                def _proj_qk_dr(wt, slab, pb):
                    # DoubleRow: lhsT [128,2,128], rhs [128,2,512]; the two
                    # slots are consecutive 128-row blocks of E (K=256/step)
                    pj = ps.tile([128, CH], F32, tag="ps", name="pj")
                    for t in range(4):
                        nc.tensor.matmul(
                            pj, wt[t][:, :, pb * 128:(pb + 1) * 128],
                            slab[:, 2 * t:2 * t + 2, :],
                            start=(t == 0), stop=(t == 3), perf_mode=DR)
                    return pj

                def proj_q(wt, slab, sc):
                    for pb in range(HP):
                        pj = _proj_qk_dr(wt, slab, pb)
                        h0, h1 = 2 * pb, 2 * pb + 1
                        cs = slice(sc * CH, (sc + 1) * CH)
                        nc.vector.tensor_copy(qz[h0][0:64, cs], pj[0:64, :])
                        nc.vector.tensor_copy(qz[h1][64:128, cs], pj[64:128, :])

                def proj_k(wt, slab, sc):
                    for pb in range(HP):
                        pj = _proj_qk_dr(wt, slab, pb)
                        nc.vector.tensor_copy(
                            kT[pb][:, sc * CH:(sc + 1) * CH], pj)

                def proj_v(wt, slab, sc):
                    for st in range(CH // 128):
                        jt = sc * (CH // 128) + st
                        pj = ps.tile([128, GD], F32, tag="ps", name="pj")
                        for et in range(8):
                            nc.tensor.matmul(
                                pj, slab[:, et, st * 128:(st + 1) * 128],
                                wt[et],
                                start=(et == 0), stop=(et == 7))
                        nc.vector.tensor_copy(
                            v_aug[:, :, jt, 0:D],
                            pj.rearrange("p (h d) -> p h d", h=HC))

                # ---------------- attention ----------------
                # per (pair p, strip it): software-pipelined scores/exp/PV
                def emit_scores(p, it, jt):
                    kdiag = jt - 4 * it
                    c0 = 128 * kdiag if kdiag > 0 else 0
                    i0 = it * 512 + c0
                    sw = ps.tile([128, 2, 512], F32, tag="ps", name="sw")
                    kt_tile = kT[p][:, jt * 128:(jt + 1) * 128]
                    nc.tensor.matmul(
                        sw[:, 0, c0:], kt_tile, qz[2 * p][:, i0:(it + 1) * 512],
                        start=True, stop=True)
                    nc.tensor.matmul(
                        sw[:, 1, c0:], kt_tile, qz[2 * p + 1][:, i0:(it + 1) * 512],
                        start=True, stop=True)
                    return sw, c0

                def emit_normalize(p, it, pxp):
                    rrow = small.tile([1, 2, 512], F32, tag="rrow", name="rrow")
                    nc.vector.reciprocal(rrow, pxp[64:65, :, :])
                    bc = small.tile([64, 2, 512], F32, tag="bc", name="bc")
                    nc.gpsimd.partition_broadcast(bc, rrow)
                    for hh in range(2):
                        nc.vector.tensor_mul(
                            xT_out[p][hh * 64:(hh + 1) * 64,
                                      it * 512:(it + 1) * 512],
                            pxp[0:64, hh, :], bc[:, hh, :])

                state = {"carry": None, "pending": None}

                def emit_strip(p, it, nxt=None):
                    h1, h2 = 2 * p, 2 * p + 1
                    jmax = 4 * it + 3
                    pxp = psx.tile([128, 2, 512], F32, tag="psx", name="pxp")
                    if state["carry"] is not None:
                        sw_cur, c0_cur = state["carry"]
                        state["carry"] = None
                    else:
                        sw_cur, c0_cur = emit_scores(p, it, 0)
                    if state["pending"] is not None:
                        emit_normalize(*state["pending"])
                        state["pending"] = None
                    for jt in range(jmax + 1):
                        if jt < jmax:
                            sw_next, c0_next = emit_scores(p, it, jt + 1)
                        elif nxt is not None:
                            # chain: next strip's jt=0 scores (kT[.. 0] is
                            # always resident) so PE has no refill bubble
                            state["carry"] = emit_scores(nxt[0], nxt[1], 0)
                        pt = ptp.tile([128, 2, 512], BF16, tag="pt", name="pt")
                        c0 = c0_cur
                        nc.scalar.activation(pt[:, :, c0:], sw_cur[:, :, c0:],
                                             EXP, scale=SCALE)
                        kdiag = jt - 4 * it
                        if kdiag >= 0:
                            cs = slice(c0, c0 + 128)
                            nc.vector.tensor_mul(pt[:, :, cs], pt[:, :, cs],
                                                 tril_t)
                        nc.tensor.matmul(
                            pxp[0:65, 0, c0:], v_aug[:, h1, jt, :], pt[:, 0, c0:],
                            start=(jt == 0), stop=(jt == jmax))
                        nc.tensor.matmul(
                            pxp[0:65, 1, c0:], v_aug[:, h2, jt, :], pt[:, 1, c0:],
                            start=(jt == 0), stop=(jt == jmax))
                        if jt < jmax:
                            sw_cur, c0_cur = sw_next, c0_next
                    state["pending"] = (p, it, pxp)

                # ---------------- fused emission ----------------
                with nc.named_scope("proj_attn"):
                    # q chunks first (strips need all of q), then per-strip
                    # k+v chunks with attention interleaved
                    chunks = [("q", xq, sc, F8) for sc in range(NCH)]
                    for it in range(IT):
                        chunks.append(("k", xk, it, F8))
                        chunks.append(("v", xv, it, BF16))
                    after = {}
                    for it in range(IT):
                        # k,v chunks 0..it cover j-tiles 0..4it+3
                        after[NCH + 2 * it + 1] = [(p, it) for p in range(HP)]
                    all_strips = [s for i in sorted(after) for s in after[i]]
                    nxt_of = {all_strips[i]: all_strips[i + 1]
                              for i in range(len(all_strips) - 1)}
                    slab_cur = transpose_chunk(chunks[0][1], chunks[0][2],
                                               chunks[0][3])
                    for i, (nm, xd, sc, dt) in enumerate(chunks):
                        if i + 1 < len(chunks):
                            nm2, xd2, sc2, dt2 = chunks[i + 1]
                            slab_next = transpose_chunk(xd2, sc2, dt2)
                        else:
                            slab_next = None
                        if nm == "q":
                            proj_q(wts["q"], slab_cur, sc)
                        elif nm == "k":
                            proj_k(wts["k"], slab_cur, sc)
                        else:
                            proj_v(wts["v"], slab_cur, sc)
                        slab_cur = slab_next
                        for (p, it) in after.get(i, ()):
                            emit_strip(p, it, nxt_of.get((p, it)))
                    if state["pending"] is not None:
                        emit_normalize(*state["pending"])
                        state["pending"] = None

                # ---------------- output projection ----------------
                with nc.named_scope("outproj"):
                    for st in range(ST):
                        po = ps.tile([128, 2, 512], F32, tag="ps", name="po")
                        for eh in range(2):
                            for kt in range(4):
                                nc.tensor.matmul(
                                    po[:, eh, :],
                                    xT_out[kt][:, st * 128:(st + 1) * 128],
                                    wot[kt * 2 + eh],
                                    start=(kt == 0), stop=(kt == 3))
                        ot = ostage.tile([128, 1024], BF16, tag="ostage",
                                         name="ot")
                        src = po.rearrange("p a b -> p (a b)")
                        if st % 2 == 0:
                            nc.vector.tensor_copy(ot, src)
                        else:
                            nc.scalar.copy(ot, src)
                        eng = nc.sync if st % 2 == 0 else nc.gpsimd
                        eng.dma_start(out=out[st * 128:(st + 1) * 128, :],
                                      in_=ot)

    nc.finalize()
    return nc


_NC = None


def _get_nc():
    global _NC
    if _NC is None:
        _NC = build_core_kernel()
    return _NC


def _tril_mask_bf16():
    import ml_dtypes
    r = np.arange(128)
    m = np.where(r[:, None] <= r[None, :], 1.0, 0.0).astype(ml_dtypes.bfloat16)
    return np.ascontiguousarray(np.repeat(m[:, None, :], 2, axis=1))


def make_in_maps(query, key, value, Wq, Wk, Wv, Wo):
    import ml_dtypes
    bf = ml_dtypes.bfloat16
    f8 = ml_dtypes.float8_e4m3
    query = np.asarray(query, np.float32)
    key = np.asarray(key, np.float32)
    value = np.asarray(value, np.float32)
    # Wq/Wk prescaled x16 so fp8 e4m3 sees sigma~0.5; exp scale folds 1/256
    Wq = (np.ascontiguousarray(np.asarray(Wq, np.float32)) * 16.0).astype(f8)
    Wk = (np.ascontiguousarray(np.asarray(Wk, np.float32)) * 16.0).astype(f8)
    Wv = np.ascontiguousarray(np.asarray(Wv, np.float32)).astype(bf)
    Wo = np.ascontiguousarray(np.asarray(Wo, np.float32)).astype(bf)
    tril_m = _tril_mask_bf16()
    xTq = np.ascontiguousarray(query.transpose(0, 2, 1)).astype(f8)
    xTk = np.ascontiguousarray(key.transpose(0, 2, 1)).astype(f8)
    xTv = np.ascontiguousarray(value.transpose(0, 2, 1)).astype(bf)
    xT = [xTq, xTk, xTv]
    in_maps = []
    for c in range(N_CORES):
        b, g = c // 2, c % 2
        cols = slice(g * GD, (g + 1) * GD)
        in_maps.append({
            "xqT": xT[0][b],
            "xkT": xT[1][b],
            "xvT": xT[2][b],
            "wq": np.ascontiguousarray(Wq[:, cols]),
            "wk": np.ascontiguousarray(Wk[:, cols]),
            "wv": np.ascontiguousarray(Wv[:, cols]),
            "wo": np.ascontiguousarray(Wo[g * GD:(g + 1) * GD, :]),
            "tril": tril_m,
        })
    return in_maps


def kernel(query, key, value, mask, Wq, Wk, Wv, Wo, **run_kwargs):
    nc = _get_nc()
    in_maps = make_in_maps(query, key, value, Wq, Wk, Wv, Wo)
    res = run_bass_kernel_spmd(nc, in_maps, core_ids=list(range(N_CORES)),
                               **run_kwargs)
    out = np.empty((B, S, E), np.float32)
    for b in range(B):
        out[b] = (res.results[2 * b]["out"].astype(np.float32)
                  + res.results[2 * b + 1]["out"].astype(np.float32))
    if run_kwargs:
        kernel.last_result = res
    return out


if __name__ == "__main__":
    rng = np.random.default_rng(0)
    q = rng.standard_normal((B, S, E), dtype=np.float32)
    k = rng.standard_normal((B, S, E), dtype=np.float32)
    v = rng.standard_normal((B, S, E), dtype=np.float32)
    sc = 1.0 / np.sqrt(E)
    Wq = rng.standard_normal((E, E), dtype=np.float32) * sc
    Wk = rng.standard_normal((E, E), dtype=np.float32) * sc
    Wv = rng.standard_normal((E, E), dtype=np.float32) * sc
    Wo = rng.standard_normal((E, E), dtype=np.float32) * sc
    o = kernel(q, k, v, None, Wq, Wk, Wv, Wo)
    print("out", o.shape, o.dtype, float(np.abs(o).mean()))



# revision 22
# speedup vs baseline: 1.4755x; 1.0291x over previous
"""Multi-head attention TRN2 Bass kernel for nn_MultiHeadAttention_77610059039245.

Problem: B=4, S=2048, E=1024, H=16 heads, d_head=64, causal mask,
scale = 1/sqrt(1024). f32 inputs/outputs; internal compute in bf16
(all matmul operands), f32 PSUM accumulation.

Sharding (8 cores): core c = (b, g): batch b = c//2, head-group g = c%2.
Each core computes heads 8g..8g+7 of batch b; host sums the two partial
output projections per batch (the Wo row-split all-reduce).

Key differences from v1 (empirically motivated by HW microbenchmarks —
real matmul cost ~= 72ns + 0.25ns*N_free, with a ~3x per-row penalty for
contraction K=64):
  - All matmul operands bf16 (halves DMA; PSUM stays f32).
  - Scores use zero-padded q tiles so the contraction is K=128 (k-pair
    tile [k_h; k_h'] x [q_h; 0]) instead of two K=64 matmuls.
  - Projection chunks are 512 wide (fewer, larger matmuls).
  - Weights are loaded into SBUF once (outside the rep loop).
  - Output DMA'd directly from PSUM (no SBUF staging copy).
"""
import numpy as np

import concourse.bass as bass
import concourse.mybir as mybir
import concourse.tile as tile
from concourse import bacc
from concourse.bass_utils import run_bass_kernel_spmd

F32 = mybir.dt.float32
BF16 = mybir.dt.bfloat16
F8 = mybir.dt.float8e4
DR = mybir.MatmulPerfMode.DoubleRow
EXP = mybir.ActivationFunctionType.Exp

B, S, E, H = 4, 2048, 1024, 16
D = 64                    # head dim
HC = 8                    # heads per core
HP = HC // 2              # head pairs per core
GD = HC * D               # per-core projected width (512)
SCALE = 1.0 / 32.0 / 256.0   # 1/sqrt(QK=1024), /256 for the 16x
                             # host-prescale of Wq and Wk (fp8 range)
N_CORES = 8
CH = 512                  # projection s-chunk
NCH = S // CH             # 4
ST = S // 128             # 16 j-tiles
IT = S // 512             # 4 i-strips


def build_core_kernel(reps=1):
    nc = bacc.Bacc("TRN2", target_bir_lowering=False)

    xq = nc.dram_tensor("xqT", [E, S], F8, kind="ExternalInput")
    xk = nc.dram_tensor("xkT", [E, S], F8, kind="ExternalInput")
    xv = nc.dram_tensor("xvT", [E, S], BF16, kind="ExternalInput")
    wq = nc.dram_tensor("wq", [E, GD], F8, kind="ExternalInput")
    wk = nc.dram_tensor("wk", [E, GD], F8, kind="ExternalInput")
    wv = nc.dram_tensor("wv", [E, GD], BF16, kind="ExternalInput")
    wo = nc.dram_tensor("wo", [GD, E], BF16, kind="ExternalInput")
    tril = nc.dram_tensor("tril", [128, 2, 128], BF16, kind="ExternalInput")
    out = nc.dram_tensor("out", [S, E], BF16, kind="ExternalOutput")

    with tile.TileContext(nc) as tc:
        with (
            tc.tile_pool(name="consts", bufs=1) as consts,
            tc.tile_pool(name="wpool", bufs=1) as wpool,
            tc.tile_pool(name="slab", bufs=4) as slabp,
            tc.tile_pool(name="qkv", bufs=1) as qkv,
            tc.tile_pool(name="pt", bufs=6) as ptp,
            tc.tile_pool(name="small", bufs=3) as small,
            tc.tile_pool(name="xto", bufs=1) as xtop,
            tc.tile_pool(name="ostage", bufs=4) as ostage,
            tc.tile_pool(name="ps", bufs=2, space="PSUM") as ps,
            tc.tile_pool(name="psx", bufs=2, space="PSUM") as psx,
        ):
            tril_t = consts.tile([128, 2, 128], BF16, name="tril_t")
            nc.sync.dma_start(out=tril_t, in_=tril[:, :, :])

            # ---- persistent tensors (allocated once; memory stable) ----
            # qz[h]: [128, S] bf16, rows (h%2)*64..+64 hold q_h^T, other
            # rows ZERO (so the K=128 scores contraction vs the k-pair
            # tile yields exactly head h's scores).
            qz = [qkv.tile([128, S], BF16, tag=f"qz{h}", name=f"qz{h}")
                  for h in range(HC)]
            kT = [qkv.tile([128, S], BF16, tag=f"kT{p}", name=f"kT{p}")
                  for p in range(HP)]
            # v augmented with a ones column (softmax denominator from PV)
            v_aug = qkv.tile([128, HC, ST, D + 1], BF16, tag="v_aug",
                             name="v_aug")
            xT_out = [xtop.tile([128, S], BF16, tag=f"xto{p}", name=f"xto{p}")
                      for p in range(HP)]

            # one-time zero fill of the pad halves of qz and the ones col
            for h in range(HC):
                dead = slice(64, 128) if h % 2 == 0 else slice(0, 64)
                nc.vector.memset(qz[h][dead, :], 0.0)
            nc.gpsimd.memset(v_aug[:, :, :, D:D + 1], 1.0)

            # ---- weights: load once into SBUF ----
            def load_w(wdram, nm, dt):
                tiles = []
                for et in range(8):
                    t = wpool.tile([128, GD], dt, tag=f"w{nm}{et}",
                                   name=f"w{nm}{et}")
                    eng = nc.sync if et % 2 == 0 else nc.gpsimd
                    eng.dma_start(out=t, in_=wdram[et * 128:(et + 1) * 128, :])
                    tiles.append(t)
                return tiles

            def load_w_paired(wdram, nm):
                # fp8 DoubleRow lhsT: [128, 2, GD], slot i = E-rows 128i
                tiles = []
                src8 = wdram.rearrange("(a p) m -> p a m", p=128)
                for t4 in range(4):
                    t = wpool.tile([128, 2, GD], F8, tag=f"w{nm}{t4}",
                                   name=f"w{nm}{t4}")
                    eng = nc.sync if t4 % 2 == 0 else nc.gpsimd
                    eng.dma_start(out=t, in_=src8[:, 2 * t4:2 * t4 + 2, :])
                    tiles.append(t)
                return tiles

            wts = {"q": load_w_paired(wq, "q"), "k": load_w_paired(wk, "k"),
                   "v": load_w(wv, "v", BF16)}
            wot = []
            for kt in range(4):
                for eh in range(2):
                    t = wpool.tile([128, GD], BF16, tag=f"wo{kt}{eh}",
                                   name=f"wo{kt}{eh}")
                    nc.gpsimd.dma_start(
                        out=t, in_=wo[kt * 128:(kt + 1) * 128,
                                      eh * 512:(eh + 1) * 512])
                    wot.append(t)

            for _rep in range(reps):
                # ---------------- projections ----------------
                def transpose_chunk(xdram, sc, dt=BF16):
                    slab = slabp.tile([128, 8, CH], dt, tag="slab",
                                      name="slab")
                    src = xdram.rearrange("(a p) s -> p a s", p=128)
                    nc.gpsimd.dma_start(
                        out=slab[:, 0:4], in_=src[:, 0:4, sc * CH:(sc + 1) * CH])
                    nc.sync.dma_start(
                        out=slab[:, 4:8], in_=src[:, 4:8, sc * CH:(sc + 1) * CH])
                    return slab

                def _proj_qk_dr(wt, slab, pb):
                    # DoubleRow: lhsT [128,2,128], rhs [128,2,512]; the two
                    # slots are consecutive 128-row blocks of E (K=256/step)
                    pj = ps.tile([128, CH], F32, tag="ps", name="pj")
                    for t in range(4):
                        nc.tensor.matmul(
                            pj, wt[t][:, :, pb * 128:(pb + 1) * 128],
                            slab[:, 2 * t:2 * t + 2, :],
                            start=(t == 0), stop=(t == 3), perf_mode=DR)
                    return pj

                def proj_q(wt, slab, sc):
                    for pb in range(HP):
                        pj = _proj_qk_dr(wt, slab, pb)
                        h0, h1 = 2 * pb, 2 * pb + 1
                        cs = slice(sc * CH, (sc + 1) * CH)
                        nc.vector.tensor_copy(qz[h0][0:64, cs], pj[0:64, :])
                        nc.vector.tensor_copy(qz[h1][64:128, cs], pj[64:128, :])

                def proj_k(wt, slab, sc):
                    for pb in range(HP):
                        pj = _proj_qk_dr(wt, slab, pb)
                        nc.vector.tensor_copy(
                            kT[pb][:, sc * CH:(sc + 1) * CH], pj)

                def proj_v(wt, slab, sc):
                    for st in range(CH // 128):
                        jt = sc * (CH // 128) + st
                        pj = ps.tile([128, GD], F32, tag="ps", name="pj")
                        for et in range(8):
                            nc.tensor.matmul(
                                pj, slab[:, et, st * 128:(st + 1) * 128],
                                wt[et],
                                start=(et == 0), stop=(et == 7))
                        nc.vector.tensor_copy(
                            v_aug[:, :, jt, 0:D],
                            pj.rearrange("p (h d) -> p h d", h=HC))

                # ---------------- attention ----------------
                # per (pair p, strip it): software-pipelined scores/exp/PV
                def emit_scores(p, it, jt):
                    kdiag = jt - 4 * it
                    c0 = 128 * kdiag if kdiag > 0 else 0
                    i0 = it * 512 + c0
                    sw = ps.tile([128, 2, 512], F32, tag="ps", name="sw")
                    kt_tile = kT[p][:, jt * 128:(jt + 1) * 128]
                    nc.tensor.matmul(
                        sw[:, 0, c0:], kt_tile, qz[2 * p][:, i0:(it + 1) * 512],
                        start=True, stop=True)
                    nc.tensor.matmul(
                        sw[:, 1, c0:], kt_tile, qz[2 * p + 1][:, i0:(it + 1) * 512],
                        start=True, stop=True)
                    return sw, c0

                def emit_normalize(p, it, pxp):
                    rrow = small.tile([1, 2, 512], F32, tag="rrow", name="rrow")
                    nc.vector.reciprocal(rrow, pxp[64:65, :, :])
                    bc = small.tile([64, 2, 512], F32, tag="bc", name="bc")
                    nc.gpsimd.partition_broadcast(bc, rrow)
                    for hh in range(2):
                        nc.vector.tensor_mul(
                            xT_out[p][hh * 64:(hh + 1) * 64,
                                      it * 512:(it + 1) * 512],
                            pxp[0:64, hh, :], bc[:, hh, :])

                state = {"carry": None, "pending": None}

                def emit_strip(p, it, nxt=None):
                    h1, h2 = 2 * p, 2 * p + 1
                    jmax = 4 * it + 3
                    pxp = psx.tile([128, 2, 512], F32, tag="psx", name="pxp")

                    def emit_pv(jt, pt, c0):
                        nc.tensor.matmul(
                            pxp[0:65, 0, c0:], v_aug[:, h1, jt, :],
                            pt[:, 0, c0:],
                            start=(jt == 0), stop=(jt == jmax))
                        nc.tensor.matmul(
                            pxp[0:65, 1, c0:], v_aug[:, h2, jt, :],
                            pt[:, 1, c0:],
                            start=(jt == 0), stop=(jt == jmax))

                    if state["carry"] is not None:
                        sw_cur, c0_cur = state["carry"]
                        state["carry"] = None
                    else:
                        sw_cur, c0_cur = emit_scores(p, it, 0)
                    if state["pending"] is not None:
                        emit_normalize(*state["pending"])
                        state["pending"] = None
                    prev = None   # (jt, pt, c0) whose PV is deferred one tile
                    for jt in range(jmax + 1):
                        # PV lags one tile AND leads the iteration: tile
                        # jt-1's exp/tril completed a full tile ago, so PE
                        # does ready work first; the scores PSUM alloc
                        # (which waits on exp(jt-1) freeing a buffer) comes
                        # after, absorbing any ACT lateness
                        if prev is not None:
                            emit_pv(*prev)
                        if jt < jmax:
                            sw_next, c0_next = emit_scores(p, it, jt + 1)
                        elif nxt is not None:
                            # chain: next strip's jt=0 scores (kT[.. 0] is
                            # always resident) so PE has no refill bubble
                            state["carry"] = emit_scores(nxt[0], nxt[1], 0)
                        pt = ptp.tile([128, 2, 512], BF16, tag="pt", name="pt")
                        c0 = c0_cur
                        nc.scalar.activation(pt[:, :, c0:], sw_cur[:, :, c0:],
                                             EXP, scale=SCALE)
                        kdiag = jt - 4 * it
                        if kdiag >= 0:
                            cs = slice(c0, c0 + 128)
                            nc.vector.tensor_mul(pt[:, :, cs], pt[:, :, cs],
                                                 tril_t)
                        prev = (jt, pt, c0)
                        if jt < jmax:
                            sw_cur, c0_cur = sw_next, c0_next
                    emit_pv(*prev)
                    state["pending"] = (p, it, pxp)

                # ---------------- fused emission ----------------
                with nc.named_scope("proj_attn"):
                    # q chunks first (strips need all of q), then per-strip
                    # k+v chunks with attention interleaved
                    chunks = [("q", xq, sc, F8) for sc in range(NCH)]
                    for it in range(IT):
                        chunks.append(("k", xk, it, F8))
                        chunks.append(("v", xv, it, BF16))
                    after = {}
                    for it in range(IT):
                        # k,v chunks 0..it cover j-tiles 0..4it+3
                        after[NCH + 2 * it + 1] = [(p, it) for p in range(HP)]
                    all_strips = [s for i in sorted(after) for s in after[i]]
                    nxt_of = {all_strips[i]: all_strips[i + 1]
                              for i in range(len(all_strips) - 1)}
                    slab_cur = transpose_chunk(chunks[0][1], chunks[0][2],
                                               chunks[0][3])
                    for i, (nm, xd, sc, dt) in enumerate(chunks):
                        if i + 1 < len(chunks):
                            nm2, xd2, sc2, dt2 = chunks[i + 1]
                            slab_next = transpose_chunk(xd2, sc2, dt2)
                        else:
                            slab_next = None
                        if nm == "q":
                            proj_q(wts["q"], slab_cur, sc)
                        elif nm == "k":
                            proj_k(wts["k"], slab_cur, sc)
                        else:
                            proj_v(wts["v"], slab_cur, sc)
                        slab_cur = slab_next
                        for (p, it) in after.get(i, ()):
                            emit_strip(p, it, nxt_of.get((p, it)))
                    if state["pending"] is not None:
                        emit_normalize(*state["pending"])
                        state["pending"] = None

                # ---------------- output projection ----------------
                with nc.named_scope("outproj"):
                    for st in range(ST):
                        po = ps.tile([128, 2, 512], F32, tag="ps", name="po")
                        for eh in range(2):
                            for kt in range(4):
                                nc.tensor.matmul(
                                    po[:, eh, :],
                                    xT_out[kt][:, st * 128:(st + 1) * 128],
                                    wot[kt * 2 + eh],
                                    start=(kt == 0), stop=(kt == 3))
                        ot = ostage.tile([128, 1024], BF16, tag="ostage",
                                         name="ot")
                        src = po.rearrange("p a b -> p (a b)")
                        if st % 2 == 0:
                            nc.vector.tensor_copy(ot, src)
                        else:
                            nc.scalar.copy(ot, src)
                        eng = nc.sync if st % 2 == 0 else nc.gpsimd
                        eng.dma_start(out=out[st * 128:(st + 1) * 128, :],
                                      in_=ot)

    nc.finalize()
    return nc


_NC = None


def _get_nc():
    global _NC
    if _NC is None:
        _NC = build_core_kernel()
    return _NC


def _tril_mask_bf16():
    import ml_dtypes
    r = np.arange(128)
    m = np.where(r[:, None] <= r[None, :], 1.0, 0.0).astype(ml_dtypes.bfloat16)
    return np.ascontiguousarray(np.repeat(m[:, None, :], 2, axis=1))


def make_in_maps(query, key, value, Wq, Wk, Wv, Wo):
    import ml_dtypes
    bf = ml_dtypes.bfloat16
    f8 = ml_dtypes.float8_e4m3
    query = np.asarray(query, np.float32)
    key = np.asarray(key, np.float32)
    value = np.asarray(value, np.float32)
    # Wq/Wk prescaled x16 so fp8 e4m3 sees sigma~0.5; exp scale folds 1/256
    Wq = (np.ascontiguousarray(np.asarray(Wq, np.float32)) * 16.0).astype(f8)
    Wk = (np.ascontiguousarray(np.asarray(Wk, np.float32)) * 16.0).astype(f8)
    Wv = np.ascontiguousarray(np.asarray(Wv, np.float32)).astype(bf)
    Wo = np.ascontiguousarray(np.asarray(Wo, np.float32)).astype(bf)
    tril_m = _tril_mask_bf16()
    xTq = np.ascontiguousarray(query.transpose(0, 2, 1)).astype(f8)
    xTk = np.ascontiguousarray(key.transpose(0, 2, 1)).astype(f8)
    xTv = np.ascontiguousarray(value.transpose(0, 2, 1)).astype(bf)
    xT = [xTq, xTk, xTv]
    in_maps = []
    for c in range(N_CORES):
        b, g = c // 2, c % 2
        cols = slice(g * GD, (g + 1) * GD)
        in_maps.append({
            "xqT": xT[0][b],
            "xkT": xT[1][b],
            "xvT": xT[2][b],
            "wq": np.ascontiguousarray(Wq[:, cols]),
            "wk": np.ascontiguousarray(Wk[:, cols]),
            "wv": np.ascontiguousarray(Wv[:, cols]),
            "wo": np.ascontiguousarray(Wo[g * GD:(g + 1) * GD, :]),
            "tril": tril_m,
        })
    return in_maps


def kernel(query, key, value, mask, Wq, Wk, Wv, Wo, **run_kwargs):
    nc = _get_nc()
    in_maps = make_in_maps(query, key, value, Wq, Wk, Wv, Wo)
    res = run_bass_kernel_spmd(nc, in_maps, core_ids=list(range(N_CORES)),
                               **run_kwargs)
    out = np.empty((B, S, E), np.float32)
    for b in range(B):
        out[b] = (res.results[2 * b]["out"].astype(np.float32)
                  + res.results[2 * b + 1]["out"].astype(np.float32))
    if run_kwargs:
        kernel.last_result = res
    return out


if __name__ == "__main__":
    rng = np.random.default_rng(0)
    q = rng.standard_normal((B, S, E), dtype=np.float32)
    k = rng.standard_normal((B, S, E), dtype=np.float32)
    v = rng.standard_normal((B, S, E), dtype=np.float32)
    sc = 1.0 / np.sqrt(E)
    Wq = rng.standard_normal((E, E), dtype=np.float32) * sc
    Wk = rng.standard_normal((E, E), dtype=np.float32) * sc
    Wv = rng.standard_normal((E, E), dtype=np.float32) * sc
    Wo = rng.standard_normal((E, E), dtype=np.float32) * sc
    o = kernel(q, k, v, None, Wq, Wk, Wv, Wo)
    print("out", o.shape, o.dtype, float(np.abs(o).mean()))



# revision 23
# speedup vs baseline: 1.5360x; 1.0410x over previous
"""Multi-head attention TRN2 Bass kernel for nn_MultiHeadAttention_77610059039245.

Problem: B=4, S=2048, E=1024, H=16 heads, d_head=64, causal mask,
scale = 1/sqrt(1024). f32 inputs/outputs; internal compute in bf16
(all matmul operands), f32 PSUM accumulation.

Sharding (8 cores): core c = (b, g): batch b = c//2, head-group g = c%2.
Each core computes heads 8g..8g+7 of batch b; host sums the two partial
output projections per batch (the Wo row-split all-reduce).

Key differences from v1 (empirically motivated by HW microbenchmarks —
real matmul cost ~= 72ns + 0.25ns*N_free, with a ~3x per-row penalty for
contraction K=64):
  - All matmul operands bf16 (halves DMA; PSUM stays f32).
  - Scores use zero-padded q tiles so the contraction is K=128 (k-pair
    tile [k_h; k_h'] x [q_h; 0]) instead of two K=64 matmuls.
  - Projection chunks are 512 wide (fewer, larger matmuls).
  - Weights are loaded into SBUF once (outside the rep loop).
  - Output DMA'd directly from PSUM (no SBUF staging copy).
"""
import numpy as np

import concourse.bass as bass
import concourse.mybir as mybir
import concourse.tile as tile
from concourse import bacc
from concourse.bass_utils import run_bass_kernel_spmd

F32 = mybir.dt.float32
BF16 = mybir.dt.bfloat16
F8 = mybir.dt.float8e4
DR = mybir.MatmulPerfMode.DoubleRow
EXP = mybir.ActivationFunctionType.Exp

B, S, E, H = 4, 2048, 1024, 16
D = 64                    # head dim
HC = 8                    # heads per core
HP = HC // 2              # head pairs per core
GD = HC * D               # per-core projected width (512)
SCALE = 1.0 / 32.0 / 256.0   # 1/sqrt(QK=1024), /256 for the 16x
                             # host-prescale of Wq and Wk (fp8 range)
N_CORES = 8
CH = 512                  # projection s-chunk
NCH = S // CH             # 4
ST = S // 128             # 16 j-tiles
IT = S // 512             # 4 i-strips


def build_core_kernel(reps=1):
    nc = bacc.Bacc("TRN2", target_bir_lowering=False)

    xq = nc.dram_tensor("xqT", [E, S], F8, kind="ExternalInput")
    xk = nc.dram_tensor("xkT", [E, S], F8, kind="ExternalInput")
    xv = nc.dram_tensor("xvT", [E, S], BF16, kind="ExternalInput")
    wq = nc.dram_tensor("wq", [E, GD], F8, kind="ExternalInput")
    wk = nc.dram_tensor("wk", [E, GD], F8, kind="ExternalInput")
    wv = nc.dram_tensor("wv", [E, GD], BF16, kind="ExternalInput")
    wo = nc.dram_tensor("wo", [GD, E], BF16, kind="ExternalInput")
    tril = nc.dram_tensor("tril", [128, 2, 128], BF16, kind="ExternalInput")
    out = nc.dram_tensor("out", [S, E], BF16, kind="ExternalOutput")

    with tile.TileContext(nc) as tc:
        with (
            tc.tile_pool(name="consts", bufs=1) as consts,
            tc.tile_pool(name="wpool", bufs=1) as wpool,
            tc.tile_pool(name="slab", bufs=4) as slabp,
            tc.tile_pool(name="qkv", bufs=1) as qkv,
            tc.tile_pool(name="pt", bufs=6) as ptp,
            tc.tile_pool(name="small", bufs=3) as small,
            tc.tile_pool(name="xto", bufs=1) as xtop,
            tc.tile_pool(name="ostage", bufs=4) as ostage,
            tc.tile_pool(name="ps", bufs=2, space="PSUM") as ps,
            tc.tile_pool(name="psx", bufs=2, space="PSUM") as psx,
        ):
            tril_t = consts.tile([128, 2, 128], BF16, name="tril_t")
            nc.sync.dma_start(out=tril_t, in_=tril[:, :, :])

            # ---- persistent tensors (allocated once; memory stable) ----
            # qz[h]: [128, S] bf16, rows (h%2)*64..+64 hold q_h^T, other
            # rows ZERO (so the K=128 scores contraction vs the k-pair
            # tile yields exactly head h's scores).
            qz = [qkv.tile([128, S], BF16, tag=f"qz{h}", name=f"qz{h}")
                  for h in range(HC)]
            kT = [qkv.tile([128, S], BF16, tag=f"kT{p}", name=f"kT{p}")
                  for p in range(HP)]
            # v augmented with a ones column (softmax denominator from PV)
            v_aug = qkv.tile([128, HC, ST, D + 1], BF16, tag="v_aug",
                             name="v_aug")
            xT_out = [xtop.tile([128, S], BF16, tag=f"xto{p}", name=f"xto{p}")
                      for p in range(HP)]

            # one-time zero fill of the pad halves of qz and the ones col
            for h in range(HC):
                dead = slice(64, 128) if h % 2 == 0 else slice(0, 64)
                nc.vector.memset(qz[h][dead, :], 0.0)
            nc.gpsimd.memset(v_aug[:, :, :, D:D + 1], 1.0)

            # ---- weights: load once into SBUF ----
            def load_w(wdram, nm, dt):
                tiles = []
                for et in range(8):
                    t = wpool.tile([128, GD], dt, tag=f"w{nm}{et}",
                                   name=f"w{nm}{et}")
                    eng = nc.sync if et % 2 == 0 else nc.gpsimd
                    eng.dma_start(out=t, in_=wdram[et * 128:(et + 1) * 128, :])
                    tiles.append(t)
                return tiles

            def load_w_paired(wdram, nm):
                # fp8 DoubleRow lhsT: [128, 2, GD], slot i = E-rows 128i
                tiles = []
                src8 = wdram.rearrange("(a p) m -> p a m", p=128)
                for t4 in range(4):
                    t = wpool.tile([128, 2, GD], F8, tag=f"w{nm}{t4}",
                                   name=f"w{nm}{t4}")
                    eng = nc.sync if t4 % 2 == 0 else nc.gpsimd
                    eng.dma_start(out=t, in_=src8[:, 2 * t4:2 * t4 + 2, :])
                    tiles.append(t)
                return tiles

            wts = {"q": load_w_paired(wq, "q"), "k": load_w_paired(wk, "k"),
                   "v": load_w(wv, "v", BF16)}
            wot = []
            for kt in range(4):
                for eh in range(2):
                    t = wpool.tile([128, GD], BF16, tag=f"wo{kt}{eh}",
                                   name=f"wo{kt}{eh}")
                    nc.gpsimd.dma_start(
                        out=t, in_=wo[kt * 128:(kt + 1) * 128,
                                      eh * 512:(eh + 1) * 512])
                    wot.append(t)

            for _rep in range(reps):
                # ---------------- projections ----------------
                def transpose_chunk(xdram, sc, dt=BF16):
                    slab = slabp.tile([128, 8, CH], dt, tag="slab",
                                      name="slab")
                    src = xdram.rearrange("(a p) s -> p a s", p=128)
                    nc.gpsimd.dma_start(
                        out=slab[:, 0:4], in_=src[:, 0:4, sc * CH:(sc + 1) * CH])
                    nc.sync.dma_start(
                        out=slab[:, 4:8], in_=src[:, 4:8, sc * CH:(sc + 1) * CH])
                    return slab

                def _proj_qk_dr(wt, slab, pb):
                    # DoubleRow: lhsT [128,2,128], rhs [128,2,512]; the two
                    # slots are consecutive 128-row blocks of E (K=256/step)
                    pj = ps.tile([128, CH], F32, tag="ps", name="pj")
                    for t in range(4):
                        nc.tensor.matmul(
                            pj, wt[t][:, :, pb * 128:(pb + 1) * 128],
                            slab[:, 2 * t:2 * t + 2, :],
                            start=(t == 0), stop=(t == 3), perf_mode=DR)
                    return pj

                def proj_q(wt, slab, sc):
                    for pb in range(HP):
                        pj = _proj_qk_dr(wt, slab, pb)
                        h0, h1 = 2 * pb, 2 * pb + 1
                        cs = slice(sc * CH, (sc + 1) * CH)
                        nc.vector.tensor_copy(qz[h0][0:64, cs], pj[0:64, :])
                        nc.vector.tensor_copy(qz[h1][64:128, cs], pj[64:128, :])

                def proj_k(wt, slab, sc):
                    for pb in range(HP):
                        pj = _proj_qk_dr(wt, slab, pb)
                        nc.vector.tensor_copy(
                            kT[pb][:, sc * CH:(sc + 1) * CH], pj)

                def proj_v(wt, slab, sc):
                    for st in range(CH // 128):
                        jt = sc * (CH // 128) + st
                        pj = ps.tile([128, GD], F32, tag="ps", name="pj")
                        for et in range(8):
                            nc.tensor.matmul(
                                pj, slab[:, et, st * 128:(st + 1) * 128],
                                wt[et],
                                start=(et == 0), stop=(et == 7))
                        nc.vector.tensor_copy(
                            v_aug[:, :, jt, 0:D],
                            pj.rearrange("p (h d) -> p h d", h=HC))

                # ---------------- attention ----------------
                # per (pair p, strip it): software-pipelined scores/exp/PV
                def emit_scores(p, it, jt):
                    kdiag = jt - 4 * it
                    c0 = 128 * kdiag if kdiag > 0 else 0
                    i0 = it * 512 + c0
                    sw = ps.tile([128, 2, 512], F32, tag="ps", name="sw")
                    kt_tile = kT[p][:, jt * 128:(jt + 1) * 128]
                    nc.tensor.matmul(
                        sw[:, 0, c0:], kt_tile, qz[2 * p][:, i0:(it + 1) * 512],
                        start=True, stop=True)
                    nc.tensor.matmul(
                        sw[:, 1, c0:], kt_tile, qz[2 * p + 1][:, i0:(it + 1) * 512],
                        start=True, stop=True)
                    return sw, c0

                def emit_normalize(p, it, pxp):
                    rrow = small.tile([1, 2, 512], F32, tag="rrow", name="rrow")
                    nc.vector.reciprocal(rrow, pxp[64:65, :, :])
                    bc = small.tile([64, 2, 512], F32, tag="bc", name="bc")
                    nc.gpsimd.partition_broadcast(bc, rrow)
                    for hh in range(2):
                        nc.vector.tensor_mul(
                            xT_out[p][hh * 64:(hh + 1) * 64,
                                      it * 512:(it + 1) * 512],
                            pxp[0:64, hh, :], bc[:, hh, :])

                state = {"carry": None, "pending": None}

                def emit_strip(p, it, nxt=None):
                    h1, h2 = 2 * p, 2 * p + 1
                    jmax = 4 * it + 3
                    pxp = psx.tile([128, 2, 512], F32, tag="psx", name="pxp")

                    def emit_pv(jt, pt, c0):
                        nc.tensor.matmul(
                            pxp[0:65, 0, c0:], v_aug[:, h1, jt, :],
                            pt[:, 0, c0:],
                            start=(jt == 0), stop=(jt == jmax))
                        nc.tensor.matmul(
                            pxp[0:65, 1, c0:], v_aug[:, h2, jt, :],
                            pt[:, 1, c0:],
                            start=(jt == 0), stop=(jt == jmax))

                    if state["carry"] is not None:
                        sw_cur, c0_cur = state["carry"]
                        state["carry"] = None
                    else:
                        sw_cur, c0_cur = emit_scores(p, it, 0)
                    if state["pending"] is not None:
                        emit_normalize(*state["pending"])
                        state["pending"] = None
                    prev = None   # (jt, pt, c0) whose PV is deferred one tile
                    for jt in range(jmax + 1):
                        if jt < jmax:
                            sw_next, c0_next = emit_scores(p, it, jt + 1)
                        elif nxt is not None:
                            # chain: next strip's jt=0 scores (kT[.. 0] is
                            # always resident) so PE has no refill bubble
                            state["carry"] = emit_scores(nxt[0], nxt[1], 0)
                        pt = ptp.tile([128, 2, 512], BF16, tag="pt", name="pt")
                        c0 = c0_cur
                        nc.scalar.activation(pt[:, :, c0:], sw_cur[:, :, c0:],
                                             EXP, scale=SCALE)
                        kdiag = jt - 4 * it
                        if kdiag >= 0:
                            cs = slice(c0, c0 + 128)
                            nc.vector.tensor_mul(pt[:, :, cs], pt[:, :, cs],
                                                 tril_t)
                        # PV lags one tile: at iteration jt PE multiplies
                        # tile jt-1, whose exp/tril completed a full tile
                        # ago, so PE never waits on the fresh ACT->DVE chain
                        if prev is not None:
                            emit_pv(*prev)
                        prev = (jt, pt, c0)
                        if jt < jmax:
                            sw_cur, c0_cur = sw_next, c0_next
                    emit_pv(*prev)
                    state["pending"] = (p, it, pxp)

                # ---------------- fused emission ----------------
                with nc.named_scope("proj_attn"):
                    # q chunks first (strips need all of q), then per-strip
                    # k+v chunks with attention interleaved
                    chunks = [("q", xq, sc, F8) for sc in range(NCH)]
                    for it in range(IT):
                        chunks.append(("k", xk, it, F8))
                        chunks.append(("v", xv, it, BF16))
                    after = {}
                    for it in range(IT):
                        # k,v chunks 0..it cover j-tiles 0..4it+3
                        after[NCH + 2 * it + 1] = [(p, it) for p in range(HP)]
                    all_strips = [s for i in sorted(after) for s in after[i]]
                    nxt_of = {all_strips[i]: all_strips[i + 1]
                              for i in range(len(all_strips) - 1)}
                    slab_cur = transpose_chunk(chunks[0][1], chunks[0][2],
                                               chunks[0][3])
                    for i, (nm, xd, sc, dt) in enumerate(chunks):
                        if i + 1 < len(chunks):
                            nm2, xd2, sc2, dt2 = chunks[i + 1]
                            slab_next = transpose_chunk(xd2, sc2, dt2)
                        else:
                            slab_next = None
                        if nm == "q":
                            proj_q(wts["q"], slab_cur, sc)
                        elif nm == "k":
                            proj_k(wts["k"], slab_cur, sc)
                        else:
                            proj_v(wts["v"], slab_cur, sc)
                        slab_cur = slab_next
                        for (p, it) in after.get(i, ()):
                            emit_strip(p, it, nxt_of.get((p, it)))
                    if state["pending"] is not None:
                        emit_normalize(*state["pending"])
                        state["pending"] = None

                # ---------------- output projection ----------------
                with nc.named_scope("outproj"):
                    for st in range(ST):
                        po = ps.tile([128, 2, 512], F32, tag="ps", name="po")
                        for eh in range(2):
                            for kt in range(4):
                                nc.tensor.matmul(
                                    po[:, eh, :],
                                    xT_out[kt][:, st * 128:(st + 1) * 128],
                                    wot[kt * 2 + eh],
                                    start=(kt == 0), stop=(kt == 3))
                        ot = ostage.tile([128, 1024], BF16, tag="ostage",
                                         name="ot")
                        src = po.rearrange("p a b -> p (a b)")
                        if st % 2 == 0:
                            nc.vector.tensor_copy(ot, src)
                        else:
                            nc.scalar.copy(ot, src)
                        eng = nc.sync if st % 2 == 0 else nc.gpsimd
                        eng.dma_start(out=out[st * 128:(st + 1) * 128, :],
                                      in_=ot)

    nc.finalize()
    return nc


_NC = None


def _get_nc():
    global _NC
    if _NC is None:
        _NC = build_core_kernel()
    return _NC


def _tril_mask_bf16():
    import ml_dtypes
    r = np.arange(128)
    m = np.where(r[:, None] <= r[None, :], 1.0, 0.0).astype(ml_dtypes.bfloat16)
    return np.ascontiguousarray(np.repeat(m[:, None, :], 2, axis=1))


def make_in_maps(query, key, value, Wq, Wk, Wv, Wo):
    import ml_dtypes
    bf = ml_dtypes.bfloat16
    f8 = ml_dtypes.float8_e4m3
    query = np.asarray(query, np.float32)
    key = np.asarray(key, np.float32)
    value = np.asarray(value, np.float32)
    # Wq/Wk prescaled x16 so fp8 e4m3 sees sigma~0.5; exp scale folds 1/256
    Wq = (np.ascontiguousarray(np.asarray(Wq, np.float32)) * 16.0).astype(f8)
    Wk = (np.ascontiguousarray(np.asarray(Wk, np.float32)) * 16.0).astype(f8)
    Wv = np.ascontiguousarray(np.asarray(Wv, np.float32)).astype(bf)
    Wo = np.ascontiguousarray(np.asarray(Wo, np.float32)).astype(bf)
    tril_m = _tril_mask_bf16()
    xTq = np.ascontiguousarray(query.transpose(0, 2, 1)).astype(f8)
    xTk = np.ascontiguousarray(key.transpose(0, 2, 1)).astype(f8)
    xTv = np.ascontiguousarray(value.transpose(0, 2, 1)).astype(bf)
    xT = [xTq, xTk, xTv]
    in_maps = []
    for c in range(N_CORES):
        b, g = c // 2, c % 2
        cols = slice(g * GD, (g + 1) * GD)
        in_maps.append({
            "xqT": xT[0][b],
            "xkT": xT[1][b],
            "xvT": xT[2][b],
            "wq": np.ascontiguousarray(Wq[:, cols]),
            "wk": np.ascontiguousarray(Wk[:, cols]),
            "wv": np.ascontiguousarray(Wv[:, cols]),
            "wo": np.ascontiguousarray(Wo[g * GD:(g + 1) * GD, :]),
            "tril": tril_m,
        })
    return in_maps


def kernel(query, key, value, mask, Wq, Wk, Wv, Wo, **run_kwargs):
    nc = _get_nc()
    in_maps = make_in_maps(query, key, value, Wq, Wk, Wv, Wo)
    res = run_bass_kernel_spmd(nc, in_maps, core_ids=list(range(N_CORES)),
                               **run_kwargs)
    out = np.empty((B, S, E), np.float32)
    for b in range(B):
        out[b] = (res.results[2 * b]["out"].astype(np.float32)
                  + res.results[2 * b + 1]["out"].astype(np.float32))
    if run_kwargs:
        kernel.last_result = res
    return out


if __name__ == "__main__":
    rng = np.random.default_rng(0)
    q = rng.standard_normal((B, S, E), dtype=np.float32)
    k = rng.standard_normal((B, S, E), dtype=np.float32)
    v = rng.standard_normal((B, S, E), dtype=np.float32)
    sc = 1.0 / np.sqrt(E)
    Wq = rng.standard_normal((E, E), dtype=np.float32) * sc
    Wk = rng.standard_normal((E, E), dtype=np.float32) * sc
    Wv = rng.standard_normal((E, E), dtype=np.float32) * sc
    Wo = rng.standard_normal((E, E), dtype=np.float32) * sc
    o = kernel(q, k, v, None, Wq, Wk, Wv, Wo)
    print("out", o.shape, o.dtype, float(np.abs(o).mean()))

